# revision 21
# baseline (speedup 1.0000x reference)
"""GAT (2-layer) + edge MLP predictor on 8 TRN2 NeuronCores.

Strategy (edge/1D graph partition parallelism):
  - Edges are sorted by dst and partitioned so core k owns dst nodes
    [k*6250, (k+1)*6250): all segment reductions are core-local.
  - Node-feature tables (f = x@W plus fused el/er attention columns) are
    built on-device; per-edge rows are fetched with indirect (gather) DMA.
  - Scatter-add (segment_sum over dst) is a one-hot matmul per 128-edge
    chunk into a PSUM accumulator for the owning 128-node tile.
  - Softmax normalization is applied per-node AFTER aggregation
    (h[n] = (sum_e ex_e f[src_e]) / denom[n]), so per-edge alpha never
    materializes.
  - Between layers, each core computes its slice of the next layer's
    table (f2 = h1 @ W2aug) locally and an AllGather replicates it.
"""

import math

import numpy as np

# ---- problem constants (hardcoded per contract) ----
N = 50000
E = 800000
FIN = 128
H = 3
D = 64
F = H * D  # 192
CLS = 10
NEG = 0.2
NCORES = 8
NPC = N // NCORES          # 6250 nodes per core
TPC = math.ceil(NPC / 128)  # 49 node tiles per core
NPPC = TPC * 128           # 6272 padded nodes per core
NPAD = NCORES * NPPC       # 50176 padded table rows
TW = 200                   # table row width: 192 f | 3 el | 3 er | 2 pad

_COMPILED = {}


def _build_program(nchunk):
    import sys
    if '/opt/trn_rl_repo' not in sys.path:
        sys.path.insert(0, '/opt/trn_rl_repo')
    import concourse.bass as bass
    import concourse.tile as tile
    from concourse import mybir

    fp32 = mybir.dt.float32
    i32 = mybir.dt.int32
    AF = mybir.ActivationFunctionType
    OP = mybir.AluOpType

    nc = bass.Bass("TRN2", target_bir_lowering=False, debug=False,
                   num_devices=NCORES, num_swdge_queues=4)

    # ---- kernel I/O ----
    nfT = nc.declare_dram_parameter("nfT", [FIN, NPAD], fp32, isOutput=False)
    W1aug = nc.declare_dram_parameter("W1aug", [FIN, TW], fp32, isOutput=False)
    W2a = nc.declare_dram_parameter("W2a", [96, TW], fp32, isOutput=False)
    W2b = nc.declare_dram_parameter("W2b", [96, TW], fp32, isOutput=False)
    Wpa = nc.declare_dram_parameter("Wpa", [96, 2 * CLS], fp32, isOutput=False)
    Wpb = nc.declare_dram_parameter("Wpb", [96, 2 * CLS], fp32, isOutput=False)
    b1bc = nc.declare_dram_parameter("b1bc", [128, F], fp32, isOutput=False)
    b2bc = nc.declare_dram_parameter("b2bc", [128, F], fp32, isOutput=False)
    bpbc = nc.declare_dram_parameter("bpbc", [128, CLS], fp32, isOutput=False)
    iota_in = nc.declare_dram_parameter("iota", [128, 128], fp32, isOutput=False)
    eye_in = nc.declare_dram_parameter("eye", [128, 128], fp32, isOutput=False)
    srcidx1 = nc.declare_dram_parameter("srcidx1", [TPC, 128, nchunk], i32, isOutput=False)
    srcidx2 = nc.declare_dram_parameter("srcidx2", [TPC, 128, nchunk], i32, isOutput=False)
    dstloc = nc.declare_dram_parameter("dstloc", [TPC, 128, nchunk], fp32, isOutput=False)
    mask3 = nc.declare_dram_parameter("mask3", [TPC, 128, 3 * nchunk], fp32, isOutput=False)
    score_out = nc.declare_dram_parameter(
        "score_out", [TPC * nchunk * 128, CLS], fp32, isOutput=True)

    with tile.TileContext(nc, num_cores=NCORES) as tc:
        with (
            tc.tile_pool(name="consts", bufs=1) as cpool,
            tc.tile_pool(name="dram", bufs=1, space="DRAM") as dpool,
            tc.tile_pool(name="meta", bufs=2) as mpool,
            tc.tile_pool(name="fsrc", bufs=2 * nchunk) as fpool,
            tc.tile_pool(name="ps", bufs=2 * nchunk) as pspool,
            tc.tile_pool(name="pg", bufs=2 * nchunk) as pgpool,
            tc.tile_pool(name="small", bufs=2) as spool,
            tc.tile_pool(name="f1work", bufs=8) as wpool,
            tc.tile_pool(name="ppose", bufs=2, space="PSUM") as tppool,
            tc.tile_pool(name="plog", bufs=2, space="PSUM") as lgpool,
            tc.tile_pool(name="pden", bufs=1, space="PSUM") as dnpool,
            tc.tile_pool(name="pout", bufs=2, space="PSUM") as oupool,
            tc.tile_pool(name="pf2", bufs=1, space="PSUM") as f2pool,
        ):
            # ---- DRAM internals ----
            F1 = dpool.tile([NPAD, TW], fp32, name="F1")
            f2sl = dpool.tile([NPPC, TW], fp32, name="f2sl")
            F2 = dpool.tile([NPAD, TW], fp32, name="F2", addr_space="Shared")
            hsdsl = dpool.tile([NPPC, 2 * CLS], fp32, name="hsdsl")
            HSD = dpool.tile([NPAD, 2 * CLS], fp32, name="HSD", addr_space="Shared")

            # ---- load constants to SBUF ----
            iota_sb = cpool.tile([128, 128], fp32, name="iota_sb")
            eye_sb = cpool.tile([128, 128], fp32, name="eye_sb")
            w1_sb = cpool.tile([FIN, TW], fp32, name="w1_sb")
            w2a_sb = cpool.tile([96, TW], fp32, name="w2a_sb")
            w2b_sb = cpool.tile([96, TW], fp32, name="w2b_sb")
            wpa_sb = cpool.tile([96, 2 * CLS], fp32, name="wpa_sb")
            wpb_sb = cpool.tile([96, 2 * CLS], fp32, name="wpb_sb")
            b1_sb = cpool.tile([128, F], fp32, name="b1_sb")
            b2_sb = cpool.tile([128, F], fp32, name="b2_sb")
            bp_sb = cpool.tile([128, CLS], fp32, name="bp_sb")
            for sb, dr in [(iota_sb, iota_in), (eye_sb, eye_in), (w1_sb, W1aug),
                           (w2a_sb, W2a), (w2b_sb, W2b), (wpa_sb, Wpa),
                           (wpb_sb, Wpb), (b1_sb, b1bc), (b2_sb, b2bc),
                           (bp_sb, bpbc)]:
                nc.sync.dma_start(sb[:], dr[:])

            # ---- phase 1: build full F1 table (replicated per core) ----
            for g in range(NPAD // 128):
                xT = wpool.tile([128, 128], fp32, name="xT", tag="xT")
                nc.sync.dma_start(xT[:], nfT[:, g * 128:(g + 1) * 128])
                f1p = f2pool.tile([128, TW], fp32, name="f1p", tag="f2p")
                nc.tensor.matmul(out=f1p[:], lhsT=xT[:], rhs=w1_sb[:],
                                 start=True, stop=True)
                f1s = wpool.tile([128, TW], fp32, name="f1s", tag="f1s")
                nc.vector.tensor_copy(f1s[:], f1p[:])
                nc.sync.dma_start(F1[g * 128:(g + 1) * 128, :], f1s[:])

            # ---- shared edge-pipeline for one GAT layer ----
            def gat_layer(table, sidx, er_src, er_cols, out_w):
                """Process the core's 49 dst tiles.  For tile t produce the
                next-layer table rows via out_w(t, hTa, hTb)."""
                for t in range(TPC):
                    six = mpool.tile([128, nchunk], i32, name="six", tag="six")
                    nc.sync.dma_start(six[:], sidx[t])
                    dlc = mpool.tile([128, nchunk], fp32, name="dlc", tag="dlc")
                    nc.sync.dma_start(dlc[:], dstloc[t])
                    m3 = mpool.tile([128, 3 * nchunk], fp32, name="m3", tag="m3")
                    nc.sync.dma_start(m3[:], mask3[t])
                    er_sb = mpool.tile([128, 3], fp32, name="er_sb", tag="er")
                    nc.sync.dma_start(
                        er_sb[:], er_src[t * 128:(t + 1) * 128, er_cols[0]:er_cols[1]])

                    logits = lgpool.tile([128, 3 * nchunk], fp32, name="logits",
                                         tag="plog")
                    fs = []
                    ps = []
                    pg = []
                    for c in range(nchunk):
                        fsc = fpool.tile([128, TW], fp32, name="fsc", tag="fsrc")
                        nc.gpsimd.indirect_dma_start(
                            out=fsc[:], out_offset=None, in_=table[:],
                            in_offset=bass.IndirectOffsetOnAxis(
                                ap=six[:, c:c + 1], axis=0))
                        psc = pspool.tile([128, 128], fp32, name="psc", tag="ps")
                        nc.vector.tensor_tensor(
                            out=psc[:], in0=dlc[:, c:c + 1].to_broadcast([128, 128]),
                            in1=iota_sb[:], op=OP.is_equal)
                        pgp = tppool.tile([128, 128], fp32, name="pgp", tag="tp")
                        nc.tensor.transpose(out=pgp[:], in_=psc[:], identity=eye_sb[:])
                        pgc = pgpool.tile([128, 128], fp32, name="pgc", tag="pg")
                        nc.vector.tensor_copy(pgc[:], pgp[:])
                        # logits[:, 3c:3c+3] = P_g^T.T @ er  (+ el_src below)
                        nc.tensor.matmul(out=logits[:, 3 * c:3 * c + 3],
                                         lhsT=pgc[:], rhs=er_sb[:],
                                         start=True, stop=True)
                        nc.vector.tensor_add(logits[:, 3 * c:3 * c + 3],
                                             logits[:, 3 * c:3 * c + 3],
                                             fsc[:, F:F + 3])
                        fs.append(fsc)
                        ps.append(psc)
                        pg.append(pgc)

                    # leaky relu + exp + pad mask over all chunks at once
                    lr = spool.tile([128, 3 * nchunk], fp32, name="lr", tag="lr")
                    nc.scalar.mul(lr[:], logits[:], NEG)
                    nc.vector.tensor_tensor(out=lr[:], in0=lr[:], in1=logits[:],
                                            op=OP.max)
                    ex = spool.tile([128, 3 * nchunk], fp32, name="ex", tag="ex")
                    nc.scalar.activation(ex[:], lr[:], AF.Exp)
                    nc.vector.tensor_mul(ex[:], ex[:], m3[:])

                    # denom accumulation
                    den = dnpool.tile([128, 3], fp32, name="den", tag="den")
                    for c in range(nchunk):
                        nc.tensor.matmul(out=den[:], lhsT=ps[c][:],
                                         rhs=ex[:, 3 * c:3 * c + 3],
                                         start=(c == 0), stop=(c == nchunk - 1))
                    dr = spool.tile([128, 3], fp32, name="dr", tag="dr")
                    nc.vector.tensor_scalar_max(dr[:], den[:], 1e-30)
                    nc.vector.reciprocal(dr[:], dr[:])

                    # pass B: weight rows by ex and scatter-accumulate
                    oup = oupool.tile([128, F], fp32, name="oup", tag="out")
                    for c in range(nchunk):
                        for h in range(H):
                            nc.vector.tensor_scalar_mul(
                                fs[c][:, h * D:(h + 1) * D],
                                fs[c][:, h * D:(h + 1) * D],
                                ex[:, 3 * c + h:3 * c + h + 1])
                        nc.tensor.matmul(out=oup[:], lhsT=ps[c][:],
                                         rhs=fs[c][:, 0:F],
                                         start=(c == 0), stop=(c == nchunk - 1))

                    # h = relu(out/denom + b)
                    h_sb = spool.tile([128, F], fp32, name="h_sb", tag="h")
                    for h in range(H):
                        nc.vector.tensor_scalar_mul(
                            h_sb[:, h * D:(h + 1) * D], oup[:, h * D:(h + 1) * D],
                            dr[:, h:h + 1])
                    nc.vector.tensor_add(h_sb[:], h_sb[:],
                                         b1_sb[:] if table is F1 else b2_sb[:])
                    nc.scalar.activation(h_sb[:], h_sb[:], AF.Relu)

                    # transpose h for the next matmul
                    hta_p = tppool.tile([96, 128], fp32, name="hta_p", tag="tp")
                    nc.tensor.transpose(out=hta_p[:], in_=h_sb[:, 0:96],
                                        identity=eye_sb[:])
                    hta = spool.tile([96, 128], fp32, name="hta", tag="hta")
                    nc.vector.tensor_copy(hta[:], hta_p[:])
                    htb_p = tppool.tile([96, 128], fp32, name="htb_p", tag="tp")
                    nc.tensor.transpose(out=htb_p[:], in_=h_sb[:, 96:F],
                                        identity=eye_sb[:])
                    htb = spool.tile([96, 128], fp32, name="htb", tag="htb")
                    nc.vector.tensor_copy(htb[:], htb_p[:])
                    out_w(t, hta, htb)

            # ---- layer 1 (writes f2sl) ----
            def l1_out(t, hta, htb):
                f2p = f2pool.tile([128, TW], fp32, name="f2p", tag="f2p")
                nc.tensor.matmul(out=f2p[:], lhsT=hta[:], rhs=w2a_sb[:],
                                 start=True, stop=False)
                nc.tensor.matmul(out=f2p[:], lhsT=htb[:], rhs=w2b_sb[:],
                                 start=False, stop=True)
                f2s = wpool.tile([128, TW], fp32, name="f2s", tag="f1s")
                nc.vector.tensor_copy(f2s[:], f2p[:])
                nc.sync.dma_start(f2sl[t * 128:(t + 1) * 128, :], f2s[:])

            gat_layer(F1, srcidx1, F1, (F + 3, F + 6), l1_out)

            # ---- allgather f2 slices into the full layer-2 table ----
            nc.gpsimd.collective_compute(
                "AllGather", mybir.AluOpType.bypass,
                replica_groups=[list(range(NCORES))],
                ins=[f2sl[:]], outs=[F2[:]])

            # ---- layer 2 (writes hsdsl) ----
            def l2_out(t, hta, htb):
                hsp = f2pool.tile([128, 2 * CLS], fp32, name="hsp", tag="f2p")
                nc.tensor.matmul(out=hsp[:], lhsT=hta[:], rhs=wpa_sb[:],
                                 start=True, stop=False)
                nc.tensor.matmul(out=hsp[:], lhsT=htb[:], rhs=wpb_sb[:],
                                 start=False, stop=True)
                hss = wpool.tile([128, 2 * CLS], fp32, name="hss", tag="f1s")
                nc.vector.tensor_copy(hss[:], hsp[:])
                nc.sync.dma_start(hsdsl[t * 128:(t + 1) * 128, :], hss[:])

            gat_layer(F2, srcidx2, f2sl, (F + 3, F + 6), l2_out)

            # ---- allgather hs/hd ----
            nc.gpsimd.collective_compute(
                "AllGather", mybir.AluOpType.bypass,
                replica_groups=[list(range(NCORES))],
                ins=[hsdsl[:]], outs=[HSD[:]])

            # ---- score pass: score_e = hs[src_e] + hd[dst_e] + bp ----
            for t in range(TPC):
                six = mpool.tile([128, nchunk], i32, name="six", tag="six")
                nc.sync.dma_start(six[:], srcidx2[t])
                dlc = mpool.tile([128, nchunk], fp32, name="dlc", tag="dlc")
                nc.sync.dma_start(dlc[:], dstloc[t])
                hd_sb = mpool.tile([128, CLS], fp32, name="hd_sb", tag="er")
                nc.sync.dma_start(hd_sb[:],
                                  hsdsl[t * 128:(t + 1) * 128, CLS:2 * CLS])
                for c in range(nchunk):
                    hsg = fpool.tile([128, 2 * CLS], fp32, name="hsg", tag="fsrc")
                    nc.gpsimd.indirect_dma_start(
                        out=hsg[:], out_offset=None, in_=HSD[:],
                        in_offset=bass.IndirectOffsetOnAxis(
                            ap=six[:, c:c + 1], axis=0))
                    psc = pspool.tile([128, 128], fp32, name="psc", tag="ps")
                    nc.vector.tensor_tensor(
                        out=psc[:], in0=dlc[:, c:c + 1].to_broadcast([128, 128]),
                        in1=iota_sb[:], op=OP.is_equal)
                    pgp = tppool.tile([128, 128], fp32, name="pgp", tag="tp")
                    nc.tensor.transpose(out=pgp[:], in_=psc[:], identity=eye_sb[:])
                    pgc = pgpool.tile([128, 128], fp32, name="pgc", tag="pg")
                    nc.vector.tensor_copy(pgc[:], pgp[:])
                    hdd = lgpool.tile([128, CLS], fp32, name="hdd", tag="plog")
                    nc.tensor.matmul(out=hdd[:], lhsT=pgc[:], rhs=hd_sb[:],
                                     start=True, stop=True)
                    sc = spool.tile([128, CLS], fp32, name="sc", tag="sc")
                    nc.vector.tensor_add(sc[:], hdd[:], hsg[:, 0:CLS])
                    nc.vector.tensor_add(sc[:], sc[:], bp_sb[:])
                    nc.sync.dma_start(
                        score_out[(t * nchunk + c) * 128:(t * nchunk + c + 1) * 128, :],
                        sc[:])

    _cap_waits(nc, mybir)
    return nc


def _cap_waits(nc, mybir, lim=1):
    """Walrus embeds at most `lim` semaphore waits per HW instruction.
    Move excess waits onto same-engine NoOps inserted just before."""
    eng_map = {
        mybir.EngineType.PE: nc.tensor,
        mybir.EngineType.DVE: nc.vector,
        mybir.EngineType.Activation: nc.scalar,
        mybir.EngineType.Pool: nc.gpsimd,
        mybir.EngineType.SP: nc.sync,
    }
    scratch = nc.main_func.blocks[-1].instructions
    for bb in nc.main_func.blocks:
        out = []
        for ins in bb.instructions:
            si = ins.sync_info
            waits = list(si.on_wait) if si is not None and si.on_wait else []
            if len(waits) > lim:
                keep = waits[-lim:]
                excess = waits[:-lim]
                eng = eng_map.get(ins.engine)
                assert eng is not None, f"no engine for {ins}"
                while excess:
                    grp, excess = excess[:lim], excess[lim:]
                    eng.nop(hint="waitsplit", nofuse=True)
                    nop = scratch.pop()
                    nop.sync_info = mybir.SyncInfo(on_wait=grp, on_update=[])
                    out.append(nop)
                ins.sync_info = mybir.SyncInfo(
                    on_wait=keep, on_update=list(si.on_update or []))
            out.append(ins)
        bb.instructions[:] = out


def _prep(src, dst, nfeats, W1, al1, ar1, b1, W2, al2, ar2, b2, Wp, bp):
    """Host-side preprocessing: edge bucketing + staged arrays."""
    src = np.asarray(src).astype(np.int64)
    dst = np.asarray(dst).astype(np.int64)
    perm = np.lexsort((src, dst))
    ds, ss = dst[perm], src[perm]
    core = ds // NPC
    loc = ds % NPC
    tt = loc // 128
    row = loc % 128
    bucket = core * TPC + tt
    counts = np.bincount(bucket, minlength=NCORES * TPC)
    nchunk = int(math.ceil(counts.max() / 128))
    spt = nchunk * 128  # slots per tile
    starts = np.zeros(NCORES * TPC + 1, np.int64)
    np.cumsum(counts, out=starts[1:])

    srcidx1 = np.zeros((NCORES, TPC, spt), np.int32)
    srcidx2 = np.zeros((NCORES, TPC, spt), np.int32)
    dstl = np.zeros((NCORES, TPC, spt), np.float32)
    mask = np.zeros((NCORES, TPC, spt), np.float32)
    orig = np.full((NCORES, TPC, spt), -1, np.int64)

    blk = ss // NPC
    r = ss % NPC
    s2_all = blk * NPPC + r
    for k in range(NCORES):
        bk = np.where(blk == k, 0, np.where(blk == 0, k, blk))
        s1_all = bk * NPPC + r
        for t in range(TPC):
            b = k * TPC + t
            n = counts[b]
            sl = slice(starts[b], starts[b + 1])
            srcidx1[k, t, :n] = s1_all[sl]
            srcidx2[k, t, :n] = s2_all[sl]
            dstl[k, t, :n] = row[sl]
            mask[k, t, :n] = 1.0
            orig[k, t, :n] = perm[sl]

    # [TPC, slots] -> [TPC, 128, nchunk]: slot j = c*128 + p
    def tr(a):
        return np.ascontiguousarray(
            a.reshape(NCORES, TPC, nchunk, 128).transpose(0, 1, 3, 2))

    srcidx1 = tr(srcidx1)
    srcidx2 = tr(srcidx2)
    dstl = tr(dstl)
    mask_t = tr(mask)
    mask3 = np.ascontiguousarray(np.repeat(mask_t, 3, axis=3))

    # node features, transposed + padded, per-core block swap (own slice first)
    nfT = np.asarray(nfeats, np.float32).T  # [128, N]
    nfp = np.zeros((FIN, NPAD), np.float32)
    for b in range(NCORES):
        nfp[:, b * NPPC:b * NPPC + NPC] = nfT[:, b * NPC:(b + 1) * NPC]
    nfTs = []
    for k in range(NCORES):
        v = nfp.copy()
        if k:
            v[:, 0:NPPC] = nfp[:, k * NPPC:(k + 1) * NPPC]
            v[:, k * NPPC:(k + 1) * NPPC] = nfp[:, 0:NPPC]
        nfTs.append(v)

    def aug(W, al, ar):
        Wg = np.zeros((W.shape[0], TW), np.float32)
        Wg[:, :F] = W
        for h in range(H):
            Wg[:, F + h] = W[:, h * D:(h + 1) * D] @ al[h]
            Wg[:, F + 3 + h] = W[:, h * D:(h + 1) * D] @ ar[h]
        return Wg

    W1aug = aug(np.asarray(W1, np.float32), al1, ar1)
    W2aug = aug(np.asarray(W2, np.float32), al2, ar2)
    Wpc = np.zeros((F, 2 * CLS), np.float32)
    Wpc[:, :CLS] = Wp[:F]
    Wpc[:, CLS:] = Wp[F:]

    consts = {
        "W1aug": W1aug,
        "W2a": np.ascontiguousarray(W2aug[:96]),
        "W2b": np.ascontiguousarray(W2aug[96:]),
        "Wpa": np.ascontiguousarray(Wpc[:96]),
        "Wpb": np.ascontiguousarray(Wpc[96:]),
        "b1bc": np.tile(np.asarray(b1, np.float32), (128, 1)),
        "b2bc": np.tile(np.asarray(b2, np.float32), (128, 1)),
        "bpbc": np.tile(np.asarray(bp, np.float32), (128, 1)),
        "iota": np.tile(np.arange(128, dtype=np.float32), (128, 1)),
        "eye": np.eye(128, dtype=np.float32),
    }
    in_maps = []
    for k in range(NCORES):
        m = dict(consts)
        m["nfT"] = nfTs[k]
        m["srcidx1"] = srcidx1[k]
        m["srcidx2"] = srcidx2[k]
        m["dstloc"] = dstl[k]
        m["mask3"] = mask3[k]
        in_maps.append(m)
    return nchunk, in_maps, orig


def kernel(src, dst, nfeats, efeats, W1, al1, ar1, b1, W2, al2, ar2, b2,
           Wp, bp, _collect=None):
    import sys
    if '/opt/trn_rl_repo' not in sys.path:
        sys.path.insert(0, '/opt/trn_rl_repo')
    from concourse.bass_utils import run_bass_kernel_spmd

    nchunk, in_maps, orig = _prep(src, dst, nfeats, W1, al1, ar1, b1,
                                  W2, al2, ar2, b2, Wp, bp)
    if nchunk not in _COMPILED:
        _COMPILED[nchunk] = _build_program(nchunk)
    nc = _COMPILED[nchunk]

    kw = dict(_collect or {})
    res = run_bass_kernel_spmd(nc, in_maps, list(range(NCORES)), **kw)
    if _collect is not None:
        _collect["results"] = res

    out = np.zeros((E, CLS), np.float32)
    for k in range(NCORES):
        sc = np.asarray(res.results[k]["score_out"])
        # device row (t*nchunk+c)*128+p  <->  host slot orig[k][t, c*128+p]
        ids = orig[k].reshape(TPC, nchunk, 128)
        sc = sc.reshape(TPC, nchunk, 128, CLS)
        m = ids >= 0
        out[ids[m]] = sc[m]
    return out

